# revision 15
# baseline (speedup 1.0000x reference)
"""CompGCNConv forward on 8 Trainium2 NeuronCores (Bass/Tile).

Strategy (edge-parallel, destination-sharded):
  - Each core owns a contiguous slice of 12500 destination nodes. Host
    routes every edge to the core that owns its destination, so no
    all-reduce of node aggregates is needed (only a [128,4] BN-stats
    all-reduce).
  - Messages: gather x[src] and rel[etype] rows (bf16) via indirect DMA,
    multiply elementwise on DVE.
  - segment_sum: one-hot scatter matmul. For each chunk of 128 edges all
    targeting one 128-node subwindow, PE computes
        aggT[d, node] += msg[e, d]^T . onehot[e, node]
    with the edge norm (and deg^-1/2 normalization) folded into the
    one-hot values. Accumulation happens in PSUM over a 512-node window.
  - Node transform: aggT windows feed (as the moving operand) matmuls
    against [w_in/3; w_out/3; diag(loop_rel)@w_loop/3], producing
    outT[dout, node] directly in the transposed layout.
  - BatchNorm: per-window partial sums/sumsq on DVE/ACT, tiny AllReduce,
    fused scale+bias on ACT. (bias input provably cancels in BN.)
  - rel_out = (rel_all @ w_rel)[:-1] computed redundantly on every core.
"""

import math
import os
import types
from contextlib import ExitStack

import ml_dtypes
import numpy as np

import concourse.bass as bass
from concourse import bacc
import concourse.tile as tile
from concourse import mybir
from concourse.bass_utils import run_bass_kernel_spmd

F32 = mybir.dt.float32
BF16 = mybir.dt.bfloat16
I32 = mybir.dt.int32
BF = ml_dtypes.bfloat16
P = 128

BN_EPS = 1e-5


def _full_cfg():
    return types.SimpleNamespace(
        n_ent=100000,
        e2=1000000,
        n_rel=200,
        d=256,
        ncores=8,
        win=512,
        gather_group=8,
    )


def _derived(cfg):
    cfg.nl = cfg.n_ent // cfg.ncores            # nodes per core
    cfg.nwin = math.ceil(cfg.nl / cfg.win)      # 512-node windows per core
    cfg.nsub_w = cfg.win // P                   # 128-node subwindows per window
    cfg.nsub = cfg.nwin * cfg.nsub_w            # subwindow slots (incl. ragged)
    cfg.dk = cfg.d // P                         # d-tiles (2 for d=256)
    return cfg


# --------------------------------------------------------------------------
# Host-side preparation: shard edges by destination, build the (identical
# across cores) chunk schedule, pack all per-core staged arrays.
# --------------------------------------------------------------------------

def _prep(cfg, inputs):
    x = np.ascontiguousarray(np.asarray(inputs["x"], dtype=np.float32))
    ei = np.asarray(inputs["edge_index"]).astype(np.int64)
    et = np.asarray(inputs["edge_type"]).astype(np.int64)
    rel = np.asarray(inputs["rel_embed"], dtype=np.float32)
    w_loop = np.asarray(inputs["w_loop"], dtype=np.float32)
    w_in = np.asarray(inputs["w_in"], dtype=np.float32)
    w_out = np.asarray(inputs["w_out"], dtype=np.float32)
    w_rel = np.asarray(inputs["w_rel"], dtype=np.float32)
    loop_rel = np.asarray(inputs["loop_rel"], dtype=np.float32)
    bn_gamma = np.asarray(inputs["bn_gamma"], dtype=np.float32)
    bn_beta = np.asarray(inputs["bn_beta"], dtype=np.float32)

    N, D, C, NL = cfg.n_ent, cfg.d, cfg.ncores, cfg.nl
    E = ei.shape[1] // 2

    halves = []
    for h, (sl,) in enumerate([(slice(0, E),), (slice(E, None),)]):
        dst = ei[0, sl]
        src = ei[1, sl]
        ty = et[sl]
        deg = np.bincount(dst, minlength=N).astype(np.float32)
        dinv = np.where(deg > 0, deg ** -0.5, 0.0).astype(np.float32)
        nrm = dinv[dst] * dinv[src]
        halves.append((dst, src, ty, nrm))

    nsub_core = math.ceil(NL / P)  # real subwindows per core (98 for full size)

    # Bucket edges by (core, subwindow) per half; compile-time chunk counts
    # must be identical across cores -> take the per-bucket max over cores.
    bucketed = []   # per half: (order, counts[C, nsub_core], starts)
    K = np.zeros((cfg.nsub, 2), dtype=np.int64)  # chunks per (sub-slot, half)
    for h, (dst, src, ty, nrm) in enumerate(halves):
        core = dst // NL
        lsub = (dst - core * NL) // P
        bucket = core * nsub_core + lsub
        order = np.argsort(bucket, kind="stable")
        counts = np.bincount(bucket, minlength=C * nsub_core).reshape(C, nsub_core)
        starts = np.zeros(C * nsub_core + 1, dtype=np.int64)
        np.cumsum(counts.reshape(-1), out=starts[1:])
        bucketed.append((order, counts, starts))
        kmax = np.ceil(counts.max(axis=0) / P).astype(np.int64)  # [nsub_core]
        kmax = np.maximum(kmax, 1)
        K[:nsub_core, h] = kmax
        K[nsub_core:, h] = 1  # ragged-tail pad slots: one all-pad chunk

    # Chunk schedule, window-major. Same for every core.
    sched = []       # (j, w, s, h, first, last)
    win_ranges = []  # per window: (j_start, j_end)
    j = 0
    for w in range(cfg.nwin):
        j0 = j
        for h in range(2):
            for s in range(cfg.nsub_w):
                ls = w * cfg.nsub_w + s
                k_n = int(K[ls, h])
                for k in range(k_n):
                    sched.append((j, w, s, h, k == 0, k == k_n - 1))
                    j += 1
        win_ranges.append((j0, j))
    nch = j

    # Per-core staged chunk arrays [128, nch].
    src_a = np.zeros((C, nch, P), dtype=np.int32)
    ety_a = np.zeros((C, nch, P), dtype=np.int32)
    dof_a = np.zeros((C, nch, P), dtype=np.float32)
    nrm_a = np.zeros((C, nch, P), dtype=np.float32)

    kk = {}
    for (jj, w, s, h, first, last) in sched:
        ls = w * cfg.nsub_w + s
        k = kk.get((ls, h), 0)
        kk[(ls, h)] = k + 1
        if ls >= nsub_core:
            continue
        order, counts, starts = bucketed[h]
        dst, src, ty, nrm = halves[h]
        for c in range(C):
            cnt = int(counts[c, ls])
            lo = k * P
            hi = min(cnt, (k + 1) * P)
            if hi <= lo:
                continue
            b = c * nsub_core + ls
            idxs = order[starts[b] + lo : starts[b] + hi]
            n = hi - lo
            src_a[c, jj, :n] = src[idxs]
            ety_a[c, jj, :n] = ty[idxs]
            dof_a[c, jj, :n] = (dst[idxs] - c * NL - ls * P).astype(np.float32)
            nrm_a[c, jj, :n] = nrm[idxs]

    # Node-transform weight pack [128, 6*2*128] f32 (or 2*dk*dk tiles).
    w_loop_eff = (w_loop * loop_rel.reshape(-1, 1)) / 3.0
    branches = [w_in / 3.0, w_out / 3.0, w_loop_eff]
    n_kt = 3 * cfg.dk
    wpack = np.zeros((P, n_kt * cfg.dk * P), dtype=np.float32)
    for kt in range(n_kt):
        b = kt // cfg.dk
        dsl = (kt % cfg.dk) * P
        for mt in range(cfg.dk):
            col = (kt * cfg.dk + mt) * P
            wpack[:, col : col + P] = branches[b][dsl : dsl + P, mt * P : (mt + 1) * P]

    # rel_out packs
    rel_all = np.vstack([rel, loop_rel]).astype(np.float32)  # [R+1, D]
    RA = rel_all.shape[0]
    relT = np.zeros((P, cfg.dk * RA), dtype=np.float32)
    for kt in range(cfg.dk):
        relT[:, kt * RA : (kt + 1) * RA] = rel_all[:, kt * P : (kt + 1) * P].T
    wrel = np.zeros((P, cfg.dk * D), dtype=np.float32)
    for kt in range(cfg.dk):
        wrel[:, kt * D : (kt + 1) * D] = w_rel[kt * P : (kt + 1) * P, :]

    gb = np.zeros((P, 2 * cfg.dk), dtype=np.float32)
    for mt in range(cfg.dk):
        gb[:, mt] = bn_gamma[mt * P : (mt + 1) * P]
        gb[:, cfg.dk + mt] = bn_beta[mt * P : (mt + 1) * P]

    x_bf = x.astype(BF)
    rel_bf = rel.astype(BF)

    iota_block = np.tile(np.arange(P, dtype=np.float32), (P, 1))
    waux = np.concatenate([wpack, relT, wrel, gb], axis=1)

    in_maps = []
    for c in range(C):
        idx_pack = np.concatenate(
            [np.ascontiguousarray(src_a[c].T), np.ascontiguousarray(ety_a[c].T)],
            axis=1,
        )
        aux_pack = np.concatenate(
            [
                np.ascontiguousarray(dof_a[c].T),
                np.ascontiguousarray(nrm_a[c].T),
                iota_block,
            ],
            axis=1,
        )
        in_maps.append(
            {
                "xg": x_bf,
                "relg": rel_bf,
                "xt": np.ascontiguousarray(x[c * NL : (c + 1) * NL].T),
                "idx": np.ascontiguousarray(idx_pack),
                "aux": np.ascontiguousarray(aux_pack),
                "waux": np.ascontiguousarray(waux),
            }
        )
    meta = types.SimpleNamespace(
        nch=nch, sched=sched, win_ranges=win_ranges, ra=RA
    )
    return in_maps, meta


# --------------------------------------------------------------------------
# Device program
# --------------------------------------------------------------------------

def _build(cfg, meta):
    N, D, NL = cfg.n_ent, cfg.d, cfg.nl
    DK = cfg.dk
    nch = meta.nch
    RA = meta.ra
    G = cfg.gather_group

    nc = bacc.Bacc(None, target_bir_lowering=False, debug=False)

    xg_d = nc.dram_tensor("xg", [N, D], BF16, kind="ExternalInput")
    relg_d = nc.dram_tensor("relg", [cfg.n_rel, D], BF16, kind="ExternalInput")
    xt_d = nc.dram_tensor("xt", [D, NL], F32, kind="ExternalInput")
    idx_d = nc.dram_tensor("idx", [P, 2 * nch], I32, kind="ExternalInput")
    aux_d = nc.dram_tensor("aux", [P, 2 * nch + P], F32, kind="ExternalInput")
    n_waux = 3 * DK * DK * P + DK * RA + DK * D + 2 * DK
    waux_d = nc.dram_tensor("waux", [P, n_waux], F32, kind="ExternalInput")

    outT_d = nc.dram_tensor("outT", [D, NL], F32, kind="ExternalOutput")
    dbg = bool(os.environ.get("KDBG"))
    if dbg:
        statdbg_d = nc.dram_tensor("statdbg", [P, 4 * DK], F32, kind="ExternalOutput")
        pre_d = nc.dram_tensor("pre", [D, NL], F32, kind="ExternalOutput")
    relout_d = nc.dram_tensor("relout", [RA, D], F32, kind="ExternalOutput")

    cc_in = nc.dram_tensor("cc_in", [P, 2 * DK], F32)
    cc_out = nc.dram_tensor("cc_out", [P, 2 * DK], F32, addr_space="Shared")

    with tile.TileContext(nc) as tc, ExitStack() as ctx:
        const = ctx.enter_context(tc.tile_pool(name="const", bufs=1))
        gpool = ctx.enter_context(tc.tile_pool(name="gath", bufs=3))
        mpool = ctx.enter_context(tc.tile_pool(name="msg", bufs=3))
        ohpool = ctx.enter_context(tc.tile_pool(name="oh", bufs=6))
        apool = ctx.enter_context(tc.tile_pool(name="aggsb", bufs=2))
        xwpool = ctx.enter_context(tc.tile_pool(name="xw", bufs=2))
        scpool = ctx.enter_context(tc.tile_pool(name="scratch", bufs=2))
        ps_a = ctx.enter_context(tc.tile_pool(name="psA", bufs=1, space="PSUM"))
        ps_o = ctx.enter_context(tc.tile_pool(name="psO", bufs=1, space="PSUM"))

        # ---- resident tiles (merged: one DMA = one semaphore each) ----
        idx_sb = const.tile([P, 2 * nch], I32, tag="idx")
        nc.sync.dma_start(out=idx_sb[:], in_=idx_d[:, :])
        aux_sb = const.tile([P, 2 * nch + P], F32, tag="aux")
        nc.sync.dma_start(out=aux_sb[:], in_=aux_d[:, :])
        waux_sb = const.tile([P, n_waux], F32, tag="waux")
        nc.sync.dma_start(out=waux_sb[:], in_=waux_d[:, :])
        src_sb = idx_sb[:, 0:nch]
        ety_sb = idx_sb[:, nch : 2 * nch]
        dof_sb = aux_sb[:, 0:nch]
        nrm_sb = aux_sb[:, nch : 2 * nch]
        iota_t = aux_sb[:, 2 * nch : 2 * nch + P]
        wpack_sb = waux_sb[:, 0 : 3 * DK * DK * P]
        o_relT = 3 * DK * DK * P
        relT_sb = waux_sb[:, o_relT : o_relT + DK * RA]
        o_wrel = o_relT + DK * RA
        wrel_sb = waux_sb[:, o_wrel : o_wrel + DK * D]
        o_gb = o_wrel + DK * D
        gb_sb = waux_sb[:, o_gb : o_gb + 2 * DK]

        out_sb = [const.tile([P, NL], F32, tag=f"out{mt}", name=f"out{mt}") for mt in range(DK)]
        s1c = [const.tile([P, cfg.nwin], F32, tag=f"s1c{mt}", name=f"s1c{mt}") for mt in range(DK)]
        s2c = [const.tile([P, cfg.nwin], F32, tag=f"s2c{mt}", name=f"s2c{mt}") for mt in range(DK)]

        # ---- rel_out (small, also warms up PE) ----
        for rt in range(math.ceil(RA / P)):
            mrows = min(P, RA - rt * P)
            prt = ps_a.tile([P, D], F32, tag="pr")
            for kt in range(DK):
                nc.tensor.matmul(
                    out=prt[:mrows, :],
                    lhsT=relT_sb[:, kt * RA + rt * P : kt * RA + rt * P + mrows],
                    rhs=wrel_sb[:, kt * D : (kt + 1) * D],
                    start=(kt == 0),
                    stop=(kt == DK - 1),
                )
            rs = scpool.tile([P, D], F32, tag="rs")
            nc.vector.tensor_copy(out=rs[:mrows, :], in_=prt[:mrows, :])
            nc.sync.dma_start(
                out=relout_d[rt * P : rt * P + mrows, :], in_=rs[:mrows, :]
            )

        variant = os.environ.get("KVAR", "")
        oh_const = None
        if variant == "nooh":
            oh_const = const.tile([P, P], BF16, tag="ohc")
            nc.vector.memset(oh_const[:], 0.0)

        # ---- main loop over node windows ----
        for w in range(cfg.nwin):
            cols_w = min(cfg.win, NL - w * cfg.win)
            pa = {}
            for h in range(2):
                for dt in range(DK):
                    pa[(h, dt)] = ps_a.tile([P, cfg.win], F32, tag=f"pa{h}{dt}", name=f"pa{h}{dt}_{w}")
            j0, j1 = meta.win_ranges[w]
            for jg in range(j0, j1, G):
                gj = min(G, j1 - jg)
                xg_t = gpool.tile([P, G * D], BF16, tag="xg")
                rg_t = gpool.tile([P, G * D], BF16, tag="rg")
                if variant == "seqdma":
                    # bisect: same bytes, sequential instead of gather
                    nc.sync.dma_start(
                        out=xg_t[:, : gj * D], in_=xg_d[0:128, 0 : gj * D]
                    )
                    nc.sync.dma_start(
                        out=rg_t[:, : gj * D], in_=xg_d[128:256, 0 : gj * D]
                    )
                elif variant == "onegather":
                    # bisect: one indirect DMA per group per table (1/G bytes)
                    nc.gpsimd.indirect_dma_start(
                        out=xg_t[:, 0:D],
                        out_offset=None,
                        in_=xg_d[:, :],
                        in_offset=bass.IndirectOffsetOnAxis(
                            ap=src_sb[:, jg : jg + 1], axis=0
                        ),
                    )
                    nc.gpsimd.indirect_dma_start(
                        out=rg_t[:, 0:D],
                        out_offset=None,
                        in_=relg_d[:, :],
                        in_offset=bass.IndirectOffsetOnAxis(
                            ap=ety_sb[:, jg : jg + 1], axis=0
                        ),
                    )
                else:
                    for jj in range(gj):
                        j = jg + jj
                        nc.gpsimd.indirect_dma_start(
                            out=xg_t[:, jj * D : (jj + 1) * D],
                            out_offset=None,
                            in_=xg_d[:, :],
                            in_offset=bass.IndirectOffsetOnAxis(
                                ap=src_sb[:, j : j + 1], axis=0
                            ),
                        )
                        nc.gpsimd.indirect_dma_start(
                            out=rg_t[:, jj * D : (jj + 1) * D],
                            out_offset=None,
                            in_=relg_d[:, :],
                            in_offset=bass.IndirectOffsetOnAxis(
                                ap=ety_sb[:, j : j + 1], axis=0
                            ),
                        )
                ms_t = mpool.tile([P, G * D], BF16, tag="ms")
                nc.vector.tensor_tensor(
                    out=ms_t[:, : gj * D],
                    in0=xg_t[:, : gj * D],
                    in1=rg_t[:, : gj * D],
                    op=mybir.AluOpType.mult,
                )
                for jj in range(gj):
                    j = jg + jj
                    (_, wj, s, h, first, last) = meta.sched[j]
                    if variant != "nooh":
                        oh_t = ohpool.tile([P, P], BF16, tag="oh")
                        nc.vector.tensor_scalar(
                            out=oh_t[:],
                            in0=iota_t,
                            scalar1=dof_sb[:, j : j + 1],
                            scalar2=nrm_sb[:, j : j + 1],
                            op0=mybir.AluOpType.is_equal,
                            op1=mybir.AluOpType.mult,
                        )
                    else:
                        oh_t = oh_const
                    if variant == "noscatmm":
                        if not first:
                            continue
                    for dt in range(DK):
                        nc.tensor.matmul(
                            out=pa[(h, dt)][:, s * P : (s + 1) * P],
                            lhsT=ms_t[:, jj * D + dt * P : jj * D + dt * P + P],
                            rhs=oh_t[:],
                            start=first,
                            stop=True if variant == "noscatmm" else last,
                        )

            # copy aggregates PSUM -> SBUF
            ag = {}
            for h in range(2):
                for dt in range(DK):
                    t = apool.tile([P, cfg.win], F32, tag=f"ag{h}{dt}")
                    nc.vector.tensor_copy(out=t[:], in_=pa[(h, dt)][:])
                    ag[(h, dt)] = t
            # x window (f32, pre-transposed on host)
            xw = []
            for dt in range(DK):
                t = xwpool.tile([P, cfg.win], F32, tag=f"xw{dt}")
                if cols_w < cfg.win:
                    nc.vector.memset(t[:, cols_w:], 0.0)
                nc.sync.dma_start(
                    out=t[:, :cols_w],
                    in_=xt_d[dt * P : (dt + 1) * P, w * cfg.win : w * cfg.win + cols_w],
                )
                xw.append(t)

            rhs_list = [ag[(0, dt)] for dt in range(DK)]
            rhs_list += [ag[(1, dt)] for dt in range(DK)]
            rhs_list += xw
            n_kt = len(rhs_list)

            po = [ps_o.tile([P, cfg.win], F32, tag=f"po{mt}", name=f"po{mt}_{w}") for mt in range(DK)]
            for kt in range(n_kt):
                for mt in range(DK):
                    nc.tensor.matmul(
                        out=po[mt][:],
                        lhsT=wpack_sb[:, (kt * DK + mt) * P : (kt * DK + mt + 1) * P],
                        rhs=rhs_list[kt][:],
                        start=(kt == 0),
                        stop=(kt == n_kt - 1),
                    )
            for mt in range(DK):
                nc.vector.tensor_copy(
                    out=out_sb[mt][:, w * cfg.win : w * cfg.win + cols_w],
                    in_=po[mt][:, :cols_w],
                )
                nc.vector.tensor_reduce(
                    out=s1c[mt][:, w : w + 1],
                    in_=po[mt][:, :cols_w],
                    axis=mybir.AxisListType.X,
                    op=mybir.AluOpType.add,
                )
                sq = scpool.tile([P, cfg.win], F32, tag="sq")
                nc.scalar.activation(
                    out=sq[:, :cols_w],
                    in_=po[mt][:, :cols_w],
                    func=mybir.ActivationFunctionType.Square,
                    accum_out=s2c[mt][:, w : w + 1],
                )

        # ---- BN stats all-reduce + normalize ----
        stat_sb = const.tile([P, 2 * DK], F32, tag="stat")
        for mt in range(DK):
            nc.vector.tensor_reduce(
                out=stat_sb[:, mt : mt + 1], in_=s1c[mt][:],
                axis=mybir.AxisListType.X, op=mybir.AluOpType.add,
            )
            nc.vector.tensor_reduce(
                out=stat_sb[:, DK + mt : DK + mt + 1], in_=s2c[mt][:],
                axis=mybir.AxisListType.X, op=mybir.AluOpType.add,
            )
        nc.sync.dma_start(out=cc_in[:, :], in_=stat_sb[:])
        nc.gpsimd.collective_compute(
            "AllReduce",
            mybir.AluOpType.add,
            replica_groups=[list(range(cfg.ncores))],
            ins=[cc_in[:, :]],
            outs=[cc_out[:, :]],
        )
        stat_rb = const.tile([P, 2 * DK], F32, tag="statrb")
        nc.sync.dma_start(out=stat_rb[:], in_=cc_out[:, :])
        if dbg:
            nc.sync.dma_start(out=statdbg_d[:, 0 : 2 * DK], in_=stat_sb[:])
            nc.sync.dma_start(out=statdbg_d[:, 2 * DK : 4 * DK], in_=stat_rb[:])
            for mt in range(DK):
                nc.sync.dma_start(
                    out=pre_d[mt * P : (mt + 1) * P, :], in_=out_sb[mt][:]
                )

        inv_n = 1.0 / float(N)
        for mt in range(DK):
            mn = scpool.tile([P, 1], F32, tag="mn")
            nc.vector.tensor_scalar_mul(mn[:], stat_rb[:, mt : mt + 1], inv_n)
            msq = scpool.tile([P, 1], F32, tag="msq")
            nc.vector.tensor_scalar_mul(msq[:], stat_rb[:, DK + mt : DK + mt + 1], inv_n)
            m2 = scpool.tile([P, 1], F32, tag="m2")
            nc.vector.tensor_tensor(
                out=m2[:], in0=mn[:], in1=mn[:], op=mybir.AluOpType.mult
            )
            var = scpool.tile([P, 1], F32, tag="var")
            nc.vector.tensor_tensor(
                out=var[:], in0=msq[:], in1=m2[:], op=mybir.AluOpType.subtract
            )
            nc.vector.tensor_scalar_add(var[:], var[:], BN_EPS)
            std = scpool.tile([P, 1], F32, tag="std")
            nc.scalar.sqrt(std[:], var[:])
            inv = scpool.tile([P, 1], F32, tag="inv")
            nc.vector.reciprocal(inv[:], std[:])
            a_t = scpool.tile([P, 1], F32, tag="a")
            nc.vector.tensor_tensor(
                out=a_t[:], in0=gb_sb[:, mt : mt + 1], in1=inv[:],
                op=mybir.AluOpType.mult,
            )
            ma = scpool.tile([P, 1], F32, tag="ma")
            nc.vector.tensor_tensor(
                out=ma[:], in0=mn[:], in1=a_t[:], op=mybir.AluOpType.mult
            )
            b_t = scpool.tile([P, 1], F32, tag="b")
            nc.vector.tensor_tensor(
                out=b_t[:], in0=gb_sb[:, DK + mt : DK + mt + 1], in1=ma[:],
                op=mybir.AluOpType.subtract,
            )
            nc.scalar.activation(
                out=out_sb[mt][:],
                in_=out_sb[mt][:],
                func=mybir.ActivationFunctionType.Identity,
                bias=b_t[:, 0:1],
                scale=a_t[:, 0:1],
            )
            nc.sync.dma_start(
                out=outT_d[mt * P : (mt + 1) * P, :], in_=out_sb[mt][:]
            )

    nc.compile()
    return nc


# --------------------------------------------------------------------------

LAST_RESULT = None


def _run(cfg, inputs, trace=False):
    global LAST_RESULT
    _derived(cfg)
    in_maps, meta = _prep(cfg, inputs)
    nc = _build(cfg, meta)
    try:
        res = run_bass_kernel_spmd(
            nc, in_maps, core_ids=list(range(cfg.ncores)), trace=trace
        )
    except ModuleNotFoundError:
        # axon NTFF profiling hook unavailable in this environment
        res = run_bass_kernel_spmd(
            nc, in_maps, core_ids=list(range(cfg.ncores)), trace=False
        )
    LAST_RESULT = res
    out = np.concatenate([np.asarray(r["outT"]).T for r in res.results], axis=0)
    rel_out = np.asarray(res.results[0]["relout"])[: cfg.n_rel]
    return np.ascontiguousarray(out, dtype=np.float32), np.ascontiguousarray(
        rel_out, dtype=np.float32
    )


def kernel(**inputs):
    # The bass runner reaches the NeuronCores through jax's axon backend; a
    # JAX_PLATFORMS=cpu pin (commonly used for the reference) would hide them.
    if os.environ.get("JAX_PLATFORMS") == "cpu":
        import jax

        try:
            plats = {d.platform for d in jax.devices()}
        except Exception:
            plats = set()
        if "axon" not in plats and not plats.intersection({"neuron"}):
            del os.environ["JAX_PLATFORMS"]
            import importlib

            import jax._src.xla_bridge as xb

            try:
                xb.backends.cache_clear()  # type: ignore[attr-defined]
            except Exception:
                pass
    cfg = _full_cfg()
    trace = bool(os.environ.get("KERNEL_TRACE"))
    return _run(cfg, inputs, trace=trace)


# revision 16
# speedup vs baseline: 1.0313x; 1.0313x over previous
"""CompGCNConv forward on 8 Trainium2 NeuronCores (Bass/Tile).

Strategy (edge-parallel, destination-sharded):
  - Each core owns a contiguous slice of 12500 destination nodes. Host
    routes every edge to the core that owns its destination, so no
    all-reduce of node aggregates is needed (only a [128,4] BN-stats
    all-reduce).
  - Messages: gather x[src] and rel[etype] rows (bf16) via indirect DMA,
    multiply elementwise on DVE.
  - segment_sum: one-hot scatter matmul. For each chunk of 128 edges all
    targeting one 128-node subwindow, PE computes
        aggT[d, node] += msg[e, d]^T . onehot[e, node]
    with the edge norm (and deg^-1/2 normalization) folded into the
    one-hot values. Accumulation happens in PSUM over a 512-node window.
  - Node transform: aggT windows feed (as the moving operand) matmuls
    against [w_in/3; w_out/3; diag(loop_rel)@w_loop/3], producing
    outT[dout, node] directly in the transposed layout.
  - BatchNorm: per-window partial sums/sumsq on DVE/ACT, tiny AllReduce,
    fused scale+bias on ACT. (bias input provably cancels in BN.)
  - rel_out = (rel_all @ w_rel)[:-1] computed redundantly on every core.
"""

import math
import os
import types
from contextlib import ExitStack

import ml_dtypes
import numpy as np

import concourse.bass as bass
from concourse import bacc
import concourse.tile as tile
from concourse import mybir
from concourse.bass_utils import run_bass_kernel_spmd

F32 = mybir.dt.float32
BF16 = mybir.dt.bfloat16
I32 = mybir.dt.int32
BF = ml_dtypes.bfloat16
P = 128

BN_EPS = 1e-5


def _full_cfg():
    return types.SimpleNamespace(
        n_ent=100000,
        e2=1000000,
        n_rel=200,
        d=256,
        ncores=8,
        win=512,
        gather_group=8,
    )


def _derived(cfg):
    cfg.nl = cfg.n_ent // cfg.ncores            # nodes per core
    cfg.nwin = math.ceil(cfg.nl / cfg.win)      # 512-node windows per core
    cfg.nsub_w = cfg.win // P                   # 128-node subwindows per window
    cfg.nsub = cfg.nwin * cfg.nsub_w            # subwindow slots (incl. ragged)
    cfg.dk = cfg.d // P                         # d-tiles (2 for d=256)
    return cfg


# --------------------------------------------------------------------------
# Host-side preparation: shard edges by destination, build the (identical
# across cores) chunk schedule, pack all per-core staged arrays.
# --------------------------------------------------------------------------

def _prep(cfg, inputs):
    x = np.ascontiguousarray(np.asarray(inputs["x"], dtype=np.float32))
    ei = np.asarray(inputs["edge_index"]).astype(np.int64)
    et = np.asarray(inputs["edge_type"]).astype(np.int64)
    rel = np.asarray(inputs["rel_embed"], dtype=np.float32)
    w_loop = np.asarray(inputs["w_loop"], dtype=np.float32)
    w_in = np.asarray(inputs["w_in"], dtype=np.float32)
    w_out = np.asarray(inputs["w_out"], dtype=np.float32)
    w_rel = np.asarray(inputs["w_rel"], dtype=np.float32)
    loop_rel = np.asarray(inputs["loop_rel"], dtype=np.float32)
    bn_gamma = np.asarray(inputs["bn_gamma"], dtype=np.float32)
    bn_beta = np.asarray(inputs["bn_beta"], dtype=np.float32)

    N, D, C, NL = cfg.n_ent, cfg.d, cfg.ncores, cfg.nl
    E = ei.shape[1] // 2

    halves = []
    for h, (sl,) in enumerate([(slice(0, E),), (slice(E, None),)]):
        dst = ei[0, sl]
        src = ei[1, sl]
        ty = et[sl]
        deg = np.bincount(dst, minlength=N).astype(np.float32)
        dinv = np.where(deg > 0, deg ** -0.5, 0.0).astype(np.float32)
        nrm = dinv[dst] * dinv[src]
        halves.append((dst, src, ty, nrm))

    nsub_core = math.ceil(NL / P)  # real subwindows per core (98 for full size)

    # Bucket edges by (core, subwindow) per half; compile-time chunk counts
    # must be identical across cores -> take the per-bucket max over cores.
    bucketed = []   # per half: (order, counts[C, nsub_core], starts)
    K = np.zeros((cfg.nsub, 2), dtype=np.int64)  # chunks per (sub-slot, half)
    for h, (dst, src, ty, nrm) in enumerate(halves):
        core = dst // NL
        lsub = (dst - core * NL) // P
        bucket = core * nsub_core + lsub
        order = np.argsort(bucket, kind="stable")
        counts = np.bincount(bucket, minlength=C * nsub_core).reshape(C, nsub_core)
        starts = np.zeros(C * nsub_core + 1, dtype=np.int64)
        np.cumsum(counts.reshape(-1), out=starts[1:])
        bucketed.append((order, counts, starts))
        kmax = np.ceil(counts.max(axis=0) / P).astype(np.int64)  # [nsub_core]
        kmax = np.maximum(kmax, 1)
        K[:nsub_core, h] = kmax
        K[nsub_core:, h] = 1  # ragged-tail pad slots: one all-pad chunk

    # Chunk schedule, window-major. Same for every core.
    sched = []       # (j, w, s, h, first, last)
    win_ranges = []  # per window: (j_start, j_end)
    j = 0
    for w in range(cfg.nwin):
        j0 = j
        for h in range(2):
            for s in range(cfg.nsub_w):
                ls = w * cfg.nsub_w + s
                k_n = int(K[ls, h])
                for k in range(k_n):
                    sched.append((j, w, s, h, k == 0, k == k_n - 1))
                    j += 1
        win_ranges.append((j0, j))
    nch = j

    # Per-core staged chunk arrays [128, nch].
    src_a = np.zeros((C, nch, P), dtype=np.int32)
    ety_a = np.zeros((C, nch, P), dtype=np.int32)
    dof_a = np.zeros((C, nch, P), dtype=np.float32)
    nrm_a = np.zeros((C, nch, P), dtype=np.float32)

    kk = {}
    for (jj, w, s, h, first, last) in sched:
        ls = w * cfg.nsub_w + s
        k = kk.get((ls, h), 0)
        kk[(ls, h)] = k + 1
        if ls >= nsub_core:
            continue
        order, counts, starts = bucketed[h]
        dst, src, ty, nrm = halves[h]
        for c in range(C):
            cnt = int(counts[c, ls])
            lo = k * P
            hi = min(cnt, (k + 1) * P)
            if hi <= lo:
                continue
            b = c * nsub_core + ls
            idxs = order[starts[b] + lo : starts[b] + hi]
            n = hi - lo
            src_a[c, jj, :n] = src[idxs]
            ety_a[c, jj, :n] = ty[idxs]
            dof_a[c, jj, :n] = (dst[idxs] - c * NL - ls * P).astype(np.float32)
            nrm_a[c, jj, :n] = nrm[idxs]

    # Node-transform weight pack [128, 6*2*128] f32 (or 2*dk*dk tiles).
    w_loop_eff = (w_loop * loop_rel.reshape(-1, 1)) / 3.0
    branches = [w_in / 3.0, w_out / 3.0, w_loop_eff]
    n_kt = 3 * cfg.dk
    wpack = np.zeros((P, n_kt * cfg.dk * P), dtype=np.float32)
    for kt in range(n_kt):
        b = kt // cfg.dk
        dsl = (kt % cfg.dk) * P
        for mt in range(cfg.dk):
            col = (kt * cfg.dk + mt) * P
            wpack[:, col : col + P] = branches[b][dsl : dsl + P, mt * P : (mt + 1) * P]

    # rel_out packs
    rel_all = np.vstack([rel, loop_rel]).astype(np.float32)  # [R+1, D]
    RA = rel_all.shape[0]
    relT = np.zeros((P, cfg.dk * RA), dtype=np.float32)
    for kt in range(cfg.dk):
        relT[:, kt * RA : (kt + 1) * RA] = rel_all[:, kt * P : (kt + 1) * P].T
    wrel = np.zeros((P, cfg.dk * D), dtype=np.float32)
    for kt in range(cfg.dk):
        wrel[:, kt * D : (kt + 1) * D] = w_rel[kt * P : (kt + 1) * P, :]

    gb = np.zeros((P, 2 * cfg.dk), dtype=np.float32)
    for mt in range(cfg.dk):
        gb[:, mt] = bn_gamma[mt * P : (mt + 1) * P]
        gb[:, cfg.dk + mt] = bn_beta[mt * P : (mt + 1) * P]

    x_bf = x.astype(BF)
    rel_bf = rel.astype(BF)

    iota_block = np.tile(np.arange(P, dtype=np.float32), (P, 1))
    waux = np.concatenate([wpack, relT, wrel, gb], axis=1)

    in_maps = []
    for c in range(C):
        idx_pack = np.concatenate(
            [np.ascontiguousarray(src_a[c].T), np.ascontiguousarray(ety_a[c].T)],
            axis=1,
        )
        aux_pack = np.concatenate(
            [
                np.ascontiguousarray(dof_a[c].T),
                np.ascontiguousarray(nrm_a[c].T),
                iota_block,
            ],
            axis=1,
        )
        in_maps.append(
            {
                "xg": x_bf,
                "relg": rel_bf,
                "xt": np.ascontiguousarray(x[c * NL : (c + 1) * NL].T),
                "idx": np.ascontiguousarray(idx_pack),
                "aux": np.ascontiguousarray(aux_pack),
                "waux": np.ascontiguousarray(waux),
            }
        )
    meta = types.SimpleNamespace(
        nch=nch, sched=sched, win_ranges=win_ranges, ra=RA
    )
    return in_maps, meta


# --------------------------------------------------------------------------
# Device program
# --------------------------------------------------------------------------

def _build(cfg, meta):
    N, D, NL = cfg.n_ent, cfg.d, cfg.nl
    DK = cfg.dk
    nch = meta.nch
    RA = meta.ra
    G = cfg.gather_group

    nc = bacc.Bacc(None, target_bir_lowering=False, debug=False)

    xg_d = nc.dram_tensor("xg", [N, D], BF16, kind="ExternalInput")
    relg_d = nc.dram_tensor("relg", [cfg.n_rel, D], BF16, kind="ExternalInput")
    xt_d = nc.dram_tensor("xt", [D, NL], F32, kind="ExternalInput")
    idx_d = nc.dram_tensor("idx", [P, 2 * nch], I32, kind="ExternalInput")
    aux_d = nc.dram_tensor("aux", [P, 2 * nch + P], F32, kind="ExternalInput")
    n_waux = 3 * DK * DK * P + DK * RA + DK * D + 2 * DK
    waux_d = nc.dram_tensor("waux", [P, n_waux], F32, kind="ExternalInput")

    outT_d = nc.dram_tensor("outT", [D, NL], F32, kind="ExternalOutput")
    dbg = bool(os.environ.get("KDBG"))
    if dbg:
        statdbg_d = nc.dram_tensor("statdbg", [P, 4 * DK], F32, kind="ExternalOutput")
        pre_d = nc.dram_tensor("pre", [D, NL], F32, kind="ExternalOutput")
    relout_d = nc.dram_tensor("relout", [RA, D], F32, kind="ExternalOutput")

    cc_in = nc.dram_tensor("cc_in", [P, 2 * DK], F32)
    cc_out = nc.dram_tensor("cc_out", [P, 2 * DK], F32, addr_space="Shared")

    with tile.TileContext(nc) as tc, ExitStack() as ctx:
        const = ctx.enter_context(tc.tile_pool(name="const", bufs=1))
        gpool = ctx.enter_context(tc.tile_pool(name="gath", bufs=3))
        mpool = ctx.enter_context(tc.tile_pool(name="msg", bufs=3))
        ohpool = ctx.enter_context(tc.tile_pool(name="oh", bufs=6))
        apool = ctx.enter_context(tc.tile_pool(name="aggsb", bufs=2))
        xwpool = ctx.enter_context(tc.tile_pool(name="xw", bufs=2))
        scpool = ctx.enter_context(tc.tile_pool(name="scratch", bufs=2))
        ps_a = ctx.enter_context(tc.tile_pool(name="psA", bufs=1, space="PSUM"))
        ps_o = ctx.enter_context(tc.tile_pool(name="psO", bufs=1, space="PSUM"))

        # ---- resident tiles (merged: one DMA = one semaphore each) ----
        idx_sb = const.tile([P, 2 * nch], I32, tag="idx")
        nc.sync.dma_start(out=idx_sb[:], in_=idx_d[:, :])
        aux_sb = const.tile([P, 2 * nch + P], F32, tag="aux")
        nc.sync.dma_start(out=aux_sb[:], in_=aux_d[:, :])
        waux_sb = const.tile([P, n_waux], F32, tag="waux")
        nc.sync.dma_start(out=waux_sb[:], in_=waux_d[:, :])
        src_sb = idx_sb[:, 0:nch]
        ety_sb = idx_sb[:, nch : 2 * nch]
        dof_sb = aux_sb[:, 0:nch]
        nrm_sb = aux_sb[:, nch : 2 * nch]
        iota_t = aux_sb[:, 2 * nch : 2 * nch + P]
        wpack_sb = waux_sb[:, 0 : 3 * DK * DK * P]
        o_relT = 3 * DK * DK * P
        relT_sb = waux_sb[:, o_relT : o_relT + DK * RA]
        o_wrel = o_relT + DK * RA
        wrel_sb = waux_sb[:, o_wrel : o_wrel + DK * D]
        o_gb = o_wrel + DK * D
        gb_sb = waux_sb[:, o_gb : o_gb + 2 * DK]

        out_sb = [const.tile([P, NL], F32, tag=f"out{mt}", name=f"out{mt}") for mt in range(DK)]
        s1c = [const.tile([P, cfg.nwin], F32, tag=f"s1c{mt}", name=f"s1c{mt}") for mt in range(DK)]
        s2c = [const.tile([P, cfg.nwin], F32, tag=f"s2c{mt}", name=f"s2c{mt}") for mt in range(DK)]

        # ---- rel_out (small, also warms up PE) ----
        for rt in range(math.ceil(RA / P)):
            mrows = min(P, RA - rt * P)
            prt = ps_a.tile([P, D], F32, tag="pr")
            for kt in range(DK):
                nc.tensor.matmul(
                    out=prt[:mrows, :],
                    lhsT=relT_sb[:, kt * RA + rt * P : kt * RA + rt * P + mrows],
                    rhs=wrel_sb[:, kt * D : (kt + 1) * D],
                    start=(kt == 0),
                    stop=(kt == DK - 1),
                )
            rs = scpool.tile([P, D], F32, tag="rs")
            nc.vector.tensor_copy(out=rs[:mrows, :], in_=prt[:mrows, :])
            nc.sync.dma_start(
                out=relout_d[rt * P : rt * P + mrows, :], in_=rs[:mrows, :]
            )

        variant = os.environ.get("KVAR", "")
        oh_const = None
        if variant == "nooh":
            oh_const = const.tile([P, P], BF16, tag="ohc")
            nc.vector.memset(oh_const[:], 0.0)

        # ---- main loop over node windows ----
        for w in range(cfg.nwin):
            cols_w = min(cfg.win, NL - w * cfg.win)
            pa = {}
            for h in range(2):
                for dt in range(DK):
                    pa[(h, dt)] = ps_a.tile([P, cfg.win], F32, tag=f"pa{h}{dt}", name=f"pa{h}{dt}_{w}")
            j0, j1 = meta.win_ranges[w]
            if variant == "nochunks":
                j1 = j0  # skip all chunk work
            for jg in range(j0, j1, G):
                gj = min(G, j1 - jg)
                xg_t = gpool.tile([P, G * D], BF16, tag="xg")
                rg_t = gpool.tile([P, G * D], BF16, tag="rg")
                if variant == "seqdma":
                    # bisect: same bytes, sequential instead of gather
                    nc.sync.dma_start(
                        out=xg_t[:, : gj * D], in_=xg_d[0:128, 0 : gj * D]
                    )
                    nc.sync.dma_start(
                        out=rg_t[:, : gj * D], in_=xg_d[128:256, 0 : gj * D]
                    )
                elif variant == "onegather":
                    # bisect: one indirect DMA per group per table (1/G bytes)
                    nc.gpsimd.indirect_dma_start(
                        out=xg_t[:, 0:D],
                        out_offset=None,
                        in_=xg_d[:, :],
                        in_offset=bass.IndirectOffsetOnAxis(
                            ap=src_sb[:, jg : jg + 1], axis=0
                        ),
                    )
                    nc.gpsimd.indirect_dma_start(
                        out=rg_t[:, 0:D],
                        out_offset=None,
                        in_=relg_d[:, :],
                        in_offset=bass.IndirectOffsetOnAxis(
                            ap=ety_sb[:, jg : jg + 1], axis=0
                        ),
                    )
                else:
                    for jj in range(gj):
                        j = jg + jj
                        nc.gpsimd.indirect_dma_start(
                            out=xg_t[:, jj * D : (jj + 1) * D],
                            out_offset=None,
                            in_=xg_d[:, :],
                            in_offset=bass.IndirectOffsetOnAxis(
                                ap=src_sb[:, j : j + 1], axis=0
                            ),
                        )
                        nc.gpsimd.indirect_dma_start(
                            out=rg_t[:, jj * D : (jj + 1) * D],
                            out_offset=None,
                            in_=relg_d[:, :],
                            in_offset=bass.IndirectOffsetOnAxis(
                                ap=ety_sb[:, j : j + 1], axis=0
                            ),
                        )
                ms_t = mpool.tile([P, G * D], BF16, tag="ms")
                nc.vector.tensor_tensor(
                    out=ms_t[:, : gj * D],
                    in0=xg_t[:, : gj * D],
                    in1=rg_t[:, : gj * D],
                    op=mybir.AluOpType.mult,
                )
                for jj in range(gj):
                    j = jg + jj
                    (_, wj, s, h, first, last) = meta.sched[j]
                    if variant != "nooh":
                        oh_t = ohpool.tile([P, P], BF16, tag="oh")
                        nc.vector.tensor_scalar(
                            out=oh_t[:],
                            in0=iota_t,
                            scalar1=dof_sb[:, j : j + 1],
                            scalar2=nrm_sb[:, j : j + 1],
                            op0=mybir.AluOpType.is_equal,
                            op1=mybir.AluOpType.mult,
                        )
                    else:
                        oh_t = oh_const
                    if variant == "noscatmm":
                        if not first:
                            continue
                    for dt in range(DK):
                        nc.tensor.matmul(
                            out=pa[(h, dt)][:, s * P : (s + 1) * P],
                            lhsT=ms_t[:, jj * D + dt * P : jj * D + dt * P + P],
                            rhs=oh_t[:],
                            start=first,
                            stop=True if variant == "noscatmm" else last,
                        )

            # copy aggregates PSUM -> SBUF
            ag = {}
            for h in range(2):
                for dt in range(DK):
                    t = apool.tile([P, cfg.win], F32, tag=f"ag{h}{dt}")
                    nc.vector.tensor_copy(out=t[:], in_=pa[(h, dt)][:])
                    ag[(h, dt)] = t
            # x window (f32, pre-transposed on host)
            xw = []
            for dt in range(DK):
                t = xwpool.tile([P, cfg.win], F32, tag=f"xw{dt}")
                if cols_w < cfg.win:
                    nc.vector.memset(t[:, cols_w:], 0.0)
                nc.sync.dma_start(
                    out=t[:, :cols_w],
                    in_=xt_d[dt * P : (dt + 1) * P, w * cfg.win : w * cfg.win + cols_w],
                )
                xw.append(t)

            rhs_list = [ag[(0, dt)] for dt in range(DK)]
            rhs_list += [ag[(1, dt)] for dt in range(DK)]
            rhs_list += xw
            n_kt = len(rhs_list)

            po = [ps_o.tile([P, cfg.win], F32, tag=f"po{mt}", name=f"po{mt}_{w}") for mt in range(DK)]
            for kt in range(n_kt):
                for mt in range(DK):
                    nc.tensor.matmul(
                        out=po[mt][:],
                        lhsT=wpack_sb[:, (kt * DK + mt) * P : (kt * DK + mt + 1) * P],
                        rhs=rhs_list[kt][:],
                        start=(kt == 0),
                        stop=(kt == n_kt - 1),
                    )
            for mt in range(DK):
                nc.vector.tensor_copy(
                    out=out_sb[mt][:, w * cfg.win : w * cfg.win + cols_w],
                    in_=po[mt][:, :cols_w],
                )
                nc.vector.tensor_reduce(
                    out=s1c[mt][:, w : w + 1],
                    in_=po[mt][:, :cols_w],
                    axis=mybir.AxisListType.X,
                    op=mybir.AluOpType.add,
                )
                sq = scpool.tile([P, cfg.win], F32, tag="sq")
                nc.scalar.activation(
                    out=sq[:, :cols_w],
                    in_=po[mt][:, :cols_w],
                    func=mybir.ActivationFunctionType.Square,
                    accum_out=s2c[mt][:, w : w + 1],
                )

        # ---- BN stats all-reduce + normalize ----
        stat_sb = const.tile([P, 2 * DK], F32, tag="stat")
        for mt in range(DK):
            nc.vector.tensor_reduce(
                out=stat_sb[:, mt : mt + 1], in_=s1c[mt][:],
                axis=mybir.AxisListType.X, op=mybir.AluOpType.add,
            )
            nc.vector.tensor_reduce(
                out=stat_sb[:, DK + mt : DK + mt + 1], in_=s2c[mt][:],
                axis=mybir.AxisListType.X, op=mybir.AluOpType.add,
            )
        stat_rb = const.tile([P, 2 * DK], F32, tag="statrb")
        if variant == "nocc":
            nc.vector.tensor_copy(out=stat_rb[:], in_=stat_sb[:])
        else:
            nc.sync.dma_start(out=cc_in[:, :], in_=stat_sb[:])
            nc.gpsimd.collective_compute(
                "AllReduce",
                mybir.AluOpType.add,
                replica_groups=[list(range(cfg.ncores))],
                ins=[cc_in[:, :]],
                outs=[cc_out[:, :]],
            )
            nc.sync.dma_start(out=stat_rb[:], in_=cc_out[:, :])
        if dbg:
            nc.sync.dma_start(out=statdbg_d[:, 0 : 2 * DK], in_=stat_sb[:])
            nc.sync.dma_start(out=statdbg_d[:, 2 * DK : 4 * DK], in_=stat_rb[:])
            for mt in range(DK):
                nc.sync.dma_start(
                    out=pre_d[mt * P : (mt + 1) * P, :], in_=out_sb[mt][:]
                )

        inv_n = 1.0 / float(N)
        for mt in range(DK):
            mn = scpool.tile([P, 1], F32, tag="mn")
            nc.vector.tensor_scalar_mul(mn[:], stat_rb[:, mt : mt + 1], inv_n)
            msq = scpool.tile([P, 1], F32, tag="msq")
            nc.vector.tensor_scalar_mul(msq[:], stat_rb[:, DK + mt : DK + mt + 1], inv_n)
            m2 = scpool.tile([P, 1], F32, tag="m2")
            nc.vector.tensor_tensor(
                out=m2[:], in0=mn[:], in1=mn[:], op=mybir.AluOpType.mult
            )
            var = scpool.tile([P, 1], F32, tag="var")
            nc.vector.tensor_tensor(
                out=var[:], in0=msq[:], in1=m2[:], op=mybir.AluOpType.subtract
            )
            nc.vector.tensor_scalar_add(var[:], var[:], BN_EPS)
            std = scpool.tile([P, 1], F32, tag="std")
            nc.scalar.sqrt(std[:], var[:])
            inv = scpool.tile([P, 1], F32, tag="inv")
            nc.vector.reciprocal(inv[:], std[:])
            a_t = scpool.tile([P, 1], F32, tag="a")
            nc.vector.tensor_tensor(
                out=a_t[:], in0=gb_sb[:, mt : mt + 1], in1=inv[:],
                op=mybir.AluOpType.mult,
            )
            ma = scpool.tile([P, 1], F32, tag="ma")
            nc.vector.tensor_tensor(
                out=ma[:], in0=mn[:], in1=a_t[:], op=mybir.AluOpType.mult
            )
            b_t = scpool.tile([P, 1], F32, tag="b")
            nc.vector.tensor_tensor(
                out=b_t[:], in0=gb_sb[:, DK + mt : DK + mt + 1], in1=ma[:],
                op=mybir.AluOpType.subtract,
            )
            nc.scalar.activation(
                out=out_sb[mt][:],
                in_=out_sb[mt][:],
                func=mybir.ActivationFunctionType.Identity,
                bias=b_t[:, 0:1],
                scale=a_t[:, 0:1],
            )
            nc.sync.dma_start(
                out=outT_d[mt * P : (mt + 1) * P, :], in_=out_sb[mt][:]
            )

    nc.compile()
    return nc


# --------------------------------------------------------------------------

LAST_RESULT = None


def _run(cfg, inputs, trace=False):
    global LAST_RESULT
    _derived(cfg)
    in_maps, meta = _prep(cfg, inputs)
    nc = _build(cfg, meta)
    try:
        res = run_bass_kernel_spmd(
            nc, in_maps, core_ids=list(range(cfg.ncores)), trace=trace
        )
    except ModuleNotFoundError:
        # axon NTFF profiling hook unavailable in this environment
        res = run_bass_kernel_spmd(
            nc, in_maps, core_ids=list(range(cfg.ncores)), trace=False
        )
    LAST_RESULT = res
    out = np.concatenate([np.asarray(r["outT"]).T for r in res.results], axis=0)
    rel_out = np.asarray(res.results[0]["relout"])[: cfg.n_rel]
    return np.ascontiguousarray(out, dtype=np.float32), np.ascontiguousarray(
        rel_out, dtype=np.float32
    )


def kernel(**inputs):
    # The bass runner reaches the NeuronCores through jax's axon backend; a
    # JAX_PLATFORMS=cpu pin (commonly used for the reference) would hide them.
    if os.environ.get("JAX_PLATFORMS") == "cpu":
        import jax

        try:
            plats = {d.platform for d in jax.devices()}
        except Exception:
            plats = set()
        if "axon" not in plats and not plats.intersection({"neuron"}):
            del os.environ["JAX_PLATFORMS"]
            import importlib

            import jax._src.xla_bridge as xb

            try:
                xb.backends.cache_clear()  # type: ignore[attr-defined]
            except Exception:
                pass
    cfg = _full_cfg()
    trace = bool(os.environ.get("KERNEL_TRACE"))
    return _run(cfg, inputs, trace=trace)
